# revision 33
# baseline (speedup 1.0000x reference)
"""Trainium2 Bass kernel for nn_Attention (dense transformer spatial attention).

Reference computation (per batch b of 4):
  X = x[b] reshaped [256, 4096]                      (4096 = 64*64 pixels)
  QKV = w_qkv @ X -> [384, 4096]; q,k,v = split(QKV) each [128, 4096]
  per head h (4 heads x 32 dims): sim = (q_h*scale)^T k_h   [4096, 4096]
  attn = softmax(sim, axis=-1); out_h = attn @ v_h^T        [4096, 32]
  H = concat_heads -> [128, 4096]; out = w_out @ H + b_out  [256, 4096]

Sharding: 8 cores = (batch b in 0..3) x (query half qh in 0..1).
Each core gets full X_b (k/v need all keys) with the query half rotated
to the front (so one compiled kernel serves both qh values), computes
attention output for its 2048 queries over all 4096 keys, and the final
projection. Host gather is concatenation + transpose.

Device algorithm (per core). The kernel streams 33.5M softmax exps per
core; throughput comes from splitting the exp work across the Scalar
and Vector engines (GpSimd cannot touch PSUM) and keeping the PE stream
cost minimal:
  - inputs land as bf16 (halves the HBM stream, ~0.4% noise, well
    inside the error budget); phase 1 (q/k/vT projections) runs before
    the main loop, copies PSUM->SBUF alternating ScalarE/DVE, chasing
    the DMA pieces which are ordered query-half first.
  - sim is computed TRANSPOSED: simT[j, i] = sum_d k[d,j] q[d,i],
    row-packed K=32 matmuls (tile_position=(32h, 0)), f32r operands
    (1 cyc/row at N>=256). Heads 0/1 go to single-bank tiles s0/s1
    (ping-pong hides the refill); heads 2/3 go to one [128, 1024]
    double-buffered tile (tag sD, 2x2 banks) so DVE consumes them in a
    SINGLE tensor_scalar per step (the per-op PSUM-access init is the
    DVE tax worth amortizing).
  - exp: heads 0/1 exact on ScalarE (Exp activation, scale folded,
    bf16 out, one [128,512] activation each). Heads 2/3 via the
    Schraudolph bit-trick on DVE: bf16 exp(x*scale) ~=
    bitcast(int16(x*(128*log2e*scale) + 127*128)), one [128,1024]
    tensor_scalar writing through an int16 view. Max elem error ~6.5%
    one-sided; softmax num/denom cancellation + diffuse attention
    bring the end-to-end error to ~6e-3 of max|out| (gate is 2e-2).
    No max-subtraction (|scale*sim| < ~8).
  - AV is computed in the [i, hd] layout: av[i-tile, (h,d)] +=
    ex_h[j, i-tile]^T @ vTaug[j, d]: the exp tile slice is the
    stationary operand, so the moving operand is vT with N=33 -> 33 PE
    cycles per matmul instead of 512. vT is augmented with a ones
    column: col 32 of each head block accumulates the softmax
    denominator for free. The 16 independent 132B accumulation groups
    per bank rely on per-byte pending-zero overwrite semantics (only a
    bank's first matmul uses start=True; group check skipped).
  - psum budget (8 banks): s0 + s1 + sD(2x2) + two av banks. The
    transpose/projection tile rotates through the SAME av pool slot at
    chunk boundaries (the brief AV-matmul backlog this causes drains
    into PE's per-step slack; exp tiles are buffered deep enough that
    the ScalarE/DVE streams never notice).
  - epilogue per 512-query chunk: reciprocal of the 16 denominators +
    normalize av -> S[i, hd] bf16 (DVE), PE transpose via identity
    matmul -> T[hd, i] bf16 psum, T copied to H sbuf (ScalarE), then
    per i-tile: projection matmul lhsT=H-tile (K=128, all 4 heads at
    once, N=256 bf16) + K=1 ones-row matmul accumulating the bias,
    PSUM->SBUF copy (ScalarE), DMA out [i, o]; the host transposes.
    Slices are emitted interleaved with the next chunk's first steps
    so the engine streams never stall on the reciprocal chain.
"""

import numpy as np

import concourse.bacc as bacc
import concourse.bass as bass
import concourse.mybir as mybir
import concourse.tile as tile
from concourse.bass_utils import run_bass_kernel_spmd

F32 = mybir.dt.float32
F32R = mybir.dt.float32r
BF16 = mybir.dt.bfloat16
I16 = mybir.dt.int16

HEADS = 4
DH = 32                      # dim per head
C = 256                      # input channels
NJ = 4096                    # keys per batch (64*64)
NI = 2048                    # queries per core (half of 4096)
JT = 128                     # j tile (partition dim of simT)
NJT = NJ // JT               # 32 j tiles
NT = 512                     # i columns per step / chunk width
NCHUNK = NI // NT            # 4
SCALE = float(DH) ** -0.5
BW = 3 * 128 + NJ            # blob256 width (w | x)
# Schraudolph bf16-exp constants: bitcast(int16(x*EA + EB)) ~ exp(x*SCALE)
EA = float(SCALE * 128.0 / np.log(2.0))
EB = float(127 * 128)
AluOp = mybir.AluOpType
Act = mybir.ActivationFunctionType


def build_kernel():
    nc = bacc.Bacc("TRN2", debug=False, num_devices=8)

    # blob256 columns (bf16): [wqkvT (384) | x rotated (4096)]
    # blob128 columns: [woutT bf16 (128 f32 words) | bias-row bf16 (128) |
    #                   identity bf16 (64) | ones-row bf16 (64)]
    blob256_d = nc.dram_tensor("blob256", [C, BW], BF16, kind="ExternalInput").ap()
    blob128_d = nc.dram_tensor("blob128", [128, 384], F32, kind="ExternalInput").ap()
    out_d = nc.dram_tensor("out_t", [NI, C], F32, kind="ExternalOutput").ap()

    with tile.TileContext(nc) as tc:
        with (
            tc.tile_pool(name="singles", bufs=1) as singles,
            tc.tile_pool(name="expp", bufs=12) as expp,
            tc.tile_pool(name="workp", bufs=4) as workp,
            tc.tile_pool(name="outp", bufs=4) as outp,
            tc.tile_pool(name="psim", bufs=1, space="PSUM") as psim,
            tc.tile_pool(name="pav", bufs=2, space="PSUM") as pav,
        ):
            # ---- resident SBUF tensors ----
            blob_sb = singles.tile([128, 2, BW], BF16)    # w|x, 2 c-tiles
            w_sb = blob_sb[:, :, 0:3 * 128]
            x_sb = blob_sb[:, :, 3 * 128:BW]
            b128_sb = singles.tile([128, 384], F32)
            woutT_bf = b128_sb[:, 0:128].bitcast(BF16)    # [128 hd, 256 o]
            bias_row = b128_sb[0:1, 128:256].bitcast(BF16)   # [1, 256]
            ident_bf = b128_sb[:, 256:320].bitcast(BF16)  # [128, 128] identity
            ones_row = b128_sb[0:1, 320:384].bitcast(BF16)   # [1, 128]
            q_sb = singles.tile([128, NI], F32R)          # rows = 4h x 32d
            k_sb = singles.tile([128, NJ], F32R)
            vT_sb = singles.tile([128, NJT, HEADS, 33], BF16)  # [j, jt, h, d|1]
            h_sb = singles.tile([128, NI], BF16)          # [hd, i] normalized

            # input DMA: 512-col pieces, query-half pieces first (interleaved
            # with the other half) so q/k groups become available in the
            # order phase 1 consumes them
            blob_v = blob256_d.rearrange("(c p) w -> p c w", c=2)
            pieces = [(0, 384),                      # weights
                      (384, 512), (896, 512),        # query half first (q0+k0)
                      (1408, 1024),
                      (384 + NI, 1024), (1408 + NI, 1024)]
            for lo, w in pieces:
                nc.sync.dma_start(out=blob_sb[:, :, lo:lo + w],
                                  in_=blob_v[:, :, lo:lo + w])
            nc.sync.dma_start(out=b128_sb, in_=blob128_d)

            # ones column of vT (col 32 of each (jt, h) block)
            nc.gpsimd.memset(vT_sb[:, :, :, 32], 1.0)

            # warm the ScalarE exp table during the DMA wait
            warm = singles.tile([1, 1], F32)
            nc.gpsimd.memset(warm, 0.0)
            nc.scalar.activation(warm, warm, Act.Exp)

            # ---- phase 1: q/k/vT projections (serial, DMA-paced) ----
            # psum rotates through the sim/av tags (all free before the main
            # loop); copies alternate ScalarE / DVE
            ph1_ctr = [0]

            def ph1_psum(name, wide=False):
                if wide:
                    return psim.tile([128, 2 * NT], F32, tag="sD",
                                     bufs=2, name=name)
                i = ph1_ctr[0] % 3
                ph1_ctr[0] += 1
                if i < 2:
                    return psim.tile([128, NT], F32, tag=f"s{i}", name=name)
                return pav.tile([128, NT], F32, tag="av", name=name)

            def ph1_copy(dst, src, on_act):
                if on_act:
                    nc.scalar.activation(dst, src, Act.Copy)
                else:
                    nc.vector.tensor_copy(dst, src)

            def emit_q_group(g, on_act):
                ps = ph1_psum("ph1q")
                for ct in range(2):
                    nc.tensor.matmul(
                        ps, lhsT=w_sb[:, ct, 0:128],
                        rhs=x_sb[:, ct, g * NT:(g + 1) * NT],
                        start=(ct == 0), stop=(ct == 1))
                ph1_copy(q_sb[:, g * NT:(g + 1) * NT], ps, on_act)

            def emit_k_group(g, on_act):
                ps = ph1_psum("ph1k")
                for ct in range(2):
                    nc.tensor.matmul(
                        ps, lhsT=w_sb[:, ct, 128:256],
                        rhs=x_sb[:, ct, g * NT:(g + 1) * NT],
                        start=(ct == 0), stop=(ct == 1))
                ph1_copy(k_sb[:, g * NT:(g + 1) * NT], ps, on_act)

            def emit_k_group2(g2, on_act):
                # 1024-wide k group in an sD-tag tile: one wide copy
                ps = ph1_psum("ph1k", wide=True)
                for half in range(2):
                    o = half * NT
                    for ct in range(2):
                        nc.tensor.matmul(
                            ps[:, o:o + NT], lhsT=w_sb[:, ct, 128:256],
                            rhs=x_sb[:, ct,
                                     g2 * 2 * NT + o:g2 * 2 * NT + o + NT],
                            start=(ct == 0), stop=(ct == 1))
                ph1_copy(k_sb[:, g2 * 2 * NT:(g2 + 1) * 2 * NT], ps, on_act)

            def emit_vt_quad(jt, on_act):
                # four j-tiles per psum tile: one wide strided copy
                ps = ph1_psum("ph1v")
                for half in range(4):
                    o = half * 128
                    nc.tensor.matmul(
                        ps[:, o:o + 128],
                        lhsT=x_sb[:, 0, (jt + half) * JT:(jt + half + 1) * JT],
                        rhs=w_sb[:, 0, 256:384], start=True, stop=False)
                    nc.tensor.matmul(
                        ps[:, o:o + 128],
                        lhsT=x_sb[:, 1, (jt + half) * JT:(jt + half + 1) * JT],
                        rhs=w_sb[:, 1, 256:384], start=False, stop=True)
                ph1_copy(vT_sb[:, jt:jt + 4, :, 0:DH],
                         ps.rearrange("p (a h d) -> p a h d", a=4, h=4),
                         on_act)

            # ordered to chase the DMA pieces; k (the DVE-critical path)
            # leads within each piece
            ph1 = [("k", 0), ("q", 0), ("k", 1), ("q", 1), ("vt", 0),
                   ("vt", 4), ("K", 1), ("q", 2), ("q", 3), ("vt", 8),
                   ("vt", 12), ("K", 2), ("K", 3), ("vt", 16),
                   ("vt", 20), ("vt", 24), ("vt", 28)]
            for i, (kind, idx) in enumerate(ph1):
                on_act = (i % 2 == 0)
                if kind == "q":
                    emit_q_group(idx, on_act)
                elif kind == "k":
                    emit_k_group(idx, on_act)
                elif kind == "K":
                    emit_k_group2(idx, on_act)
                else:
                    emit_vt_quad(idx, on_act)

            # ---- main loop ----
            pending_av = None
            pending_epi = []

            def emit_av(ex_of_h, av_views, jt):
                first = (jt == 0)
                last = (jt == NJT - 1)
                for it in range(4):
                    av = av_views[it // 2]
                    for h in range(HEADS):
                        nc.tensor.matmul(
                            av[:, it % 2, h, :],
                            lhsT=ex_of_h(h, it),
                            rhs=vT_sb[:, jt, h, :],
                            start=(first and it % 2 == 0 and h == 0),
                            stop=(last and it % 2 == 1 and h == 3),
                            skip_group_check=True)

            def make_epilogue(chunk, av_tiles, av_views):
                co = chunk * NT
                den = workp.tile([128, 16], F32, tag="den")
                rc = workp.tile([128, 16], F32, tag="rc")
                s_t = workp.tile([128, 4, 128], BF16, tag="s")
                # projection psum lives in the unused tail columns of the av
                # tiles: avA[264:512] (248 cols, exactly to the bank edge) +
                # avB[264:272] (8 cols)
                pj_a = av_tiles[0][:, 264:512]
                pj_b = av_tiles[1][:, 264:272]

                def p_recip():
                    for i, av in enumerate(av_views):
                        nc.vector.tensor_copy(
                            den[:, i * 8:(i + 1) * 8],
                            av[:, :, :, 32].rearrange("p a b -> p (a b)"))
                    nc.vector.reciprocal(out=rc, in_=den)

                def p_norm_tile(i):
                    # one wide op per av tile: rc broadcast over d via a
                    # stride-0 trailing dim
                    av = av_views[i]
                    rcb = rc[:, i * 8:(i + 1) * 8].rearrange(
                        "p (a h) -> p a h", a=2).broadcast_to((128, 2, 4, DH))
                    nc.vector.tensor_tensor(
                        out=s_t[:, 2 * i:2 * i + 2, :].rearrange(
                            "p a (h d) -> p a h d", h=4),
                        in0=av[:, :, :, 0:DH],
                        in1=rcb, op=AluOp.mult)

                def p_trans():
                    # S[i, hd] -> H[hd, i] via the DMA transpose XBAR. In the
                    # final flush the transposes sit on the critical path and
                    # ScalarE is idle, so issue them from its queue there to
                    # avoid serializing behind the SP output DMAs
                    last_c = (chunk == NCHUNK - 1)
                    for it in range(4):
                        eng = (nc.scalar if it % 2 == 0 else nc.sync) \
                            if last_c else nc.sync
                        eng.dma_start(
                            out=h_sb[:, co + it * 128:co + (it + 1) * 128],
                            in_=s_t[:, it, :], transpose=True)

                def p_pj(it):
                    io = co + it * 128
                    nc.tensor.matmul(pj_a, lhsT=h_sb[:, io:io + 128],
                                     rhs=woutT_bf[:, 0:248],
                                     start=True, stop=False)
                    nc.tensor.matmul(pj_a, lhsT=ones_row,
                                     rhs=bias_row[:, 0:248],
                                     start=False, stop=True)
                    nc.tensor.matmul(pj_b, lhsT=h_sb[:, io:io + 128],
                                     rhs=woutT_bf[:, 248:256],
                                     start=True, stop=False)
                    nc.tensor.matmul(pj_b, lhsT=ones_row,
                                     rhs=bias_row[:, 248:256],
                                     start=False, stop=True)

                def p_ot(it):
                    io = co + it * 128
                    ot = outp.tile([128, C], F32, tag="out")
                    if it % 2 == 0 and chunk != NCHUNK - 1:
                        nc.scalar.activation(ot[:, 0:248], pj_a, Act.Copy)
                        nc.scalar.activation(ot[:, 248:256], pj_b, Act.Copy)
                    else:
                        nc.vector.tensor_copy(ot[:, 0:248], pj_a)
                        nc.vector.tensor_copy(ot[:, 248:256], pj_b)
                    nc.sync.dma_start(out=out_d[io:io + 128, :], in_=ot)

                if chunk == NCHUNK - 1:
                    # the sD banks are free after the last step: use them as
                    # a second projection region so the final chain runs
                    # two-wide instead of fully serial
                    pjx = psim.tile([128, 2 * NT], F32, tag="sD", bufs=2,
                                    name="pjx")
                    pj2 = (pjx[:, 0:248], pjx[:, 256:264])

                    def p_pj2(it):
                        io = co + it * 128
                        for (dst, lo, hi) in ((pj2[0], 0, 248),
                                              (pj2[1], 248, 256)):
                            nc.tensor.matmul(dst, lhsT=h_sb[:, io:io + 128],
                                             rhs=woutT_bf[:, lo:hi],
                                             start=True, stop=False)
                            nc.tensor.matmul(dst, lhsT=ones_row,
                                             rhs=bias_row[:, lo:hi],
                                             start=False, stop=True)

                    def p_ot2(it):
                        io = co + it * 128
                        ot = outp.tile([128, C], F32, tag="out")
                        nc.vector.tensor_copy(ot[:, 0:248], pj2[0])
                        nc.vector.tensor_copy(ot[:, 248:256], pj2[1])
                        nc.gpsimd.dma_start(out=out_d[io:io + 128, :], in_=ot)

                    return [p_recip,
                            lambda: (p_norm_tile(0), p_norm_tile(1)),
                            p_trans,
                            lambda: (p_pj(0), p_pj2(1)),
                            lambda: (p_ot(0), p_ot2(1), p_pj(2), p_pj2(3)),
                            lambda: (p_ot(2), p_ot2(3))]

                nop = lambda: None
                return [p_recip,
                        lambda: (p_norm_tile(0), p_norm_tile(1)),
                        p_trans, nop,
                        lambda: p_pj(0), nop, lambda: (p_ot(0), p_pj(1)),
                        nop, lambda: (p_ot(1), p_pj(2)), nop,
                        lambda: (p_ot(2), p_pj(3)), nop, lambda: p_ot(3)]

            for chunk in range(NCHUNK):
                co = chunk * NT
                av_tiles = []
                av_views = []
                for nm in ("avA", "avB"):
                    t = pav.tile([128, 512], F32, tag="av", name=nm)
                    av_tiles.append(t)
                    av_views.append(
                        t[:, 0:264].rearrange("p (a h d) -> p a h d", a=2, h=4))
                deferred = []
                for jt in range(NJT):
                    s0 = psim.tile([128, NT], F32, tag="s0")
                    s1 = psim.tile([128, NT], F32, tag="s1")
                    sD = psim.tile([128, 2 * NT], F32, tag="sD", bufs=2)
                    targets = (s0, s1, sD[:, 0:NT], sD[:, NT:2 * NT])
                    for h in range(HEADS):
                        nc.tensor.matmul(
                            targets[h],
                            lhsT=k_sb[h * DH:(h + 1) * DH,
                                      jt * JT:(jt + 1) * JT],
                            rhs=q_sb[h * DH:(h + 1) * DH, co:co + NT],
                            start=True, stop=True,
                            tile_position=(h * DH, 0))
                    e0 = expp.tile([128, NT], BF16, tag="e0")
                    e1 = expp.tile([128, NT], BF16, tag="e1")
                    e23 = expp.tile([128, 2 * NT], BF16, tag="e23")
                    nc.scalar.activation(e0, s0, Act.Exp, scale=SCALE)
                    nc.scalar.activation(e1, s1, Act.Exp, scale=SCALE)
                    nc.vector.tensor_scalar(
                        out=e23.bitcast(I16), in0=sD,
                        scalar1=EA, scalar2=EB, op0=AluOp.mult, op1=AluOp.add)

                    def ex_of_h(h, it, e0=e0, e1=e1, e23=e23):
                        if h == 0:
                            return e0[:, it * 128:(it + 1) * 128]
                        if h == 1:
                            return e1[:, it * 128:(it + 1) * 128]
                        off = (h - 2) * NT + it * 128
                        return e23[:, off:off + 128]

                    # AV matmuls into the av tiles must not enter the PE
                    # wait queue while the previous chunk's projection chain
                    # still reads those banks: buffer the first steps' AVs
                    # and flush once the epilogue window has passed
                    if pending_av is not None:
                        if chunk > 0 and jt <= 8:
                            deferred.append(pending_av)
                        else:
                            for _ in range(2):
                                if deferred:
                                    emit_av(*deferred.pop(0))
                            emit_av(*pending_av)
                    pending_av = (ex_of_h, av_views, jt)

                    if pending_epi:
                        pending_epi.pop(0)()

                while deferred:
                    emit_av(*deferred.pop(0))
                emit_av(*pending_av)
                pending_av = None
                pending_epi = make_epilogue(chunk, av_tiles, av_views)
            while pending_epi:
                pending_epi.pop(0)()

    nc.compile()
    return nc


_NC = None


def _get_nc():
    global _NC
    if _NC is None:
        _NC = build_kernel()
    return _NC


def make_in_maps(x, w_qkv, w_out, b_out):
    import ml_dtypes
    x = np.ascontiguousarray(np.asarray(x, dtype=np.float32))
    w_qkv = np.asarray(w_qkv, dtype=np.float32)
    w_out = np.asarray(w_out, dtype=np.float32)
    b_out = np.asarray(b_out, dtype=np.float32)

    wqkvT = w_qkv.T                                       # [256, 384]
    woutT = w_out.T                                       # [128 hd, 256 o]

    def pack_bf16(a):
        bf = a.astype(ml_dtypes.bfloat16).view(np.uint16)
        lo = bf[:, 0::2].astype(np.uint32)
        hi = bf[:, 1::2].astype(np.uint32)
        return (lo | (hi << 16)).view(np.float32)

    bias_ones = np.zeros((128, 256 + 128), np.float32)
    bias_ones[0, 0:256] = b_out
    bias_ones[0, 256:384] = 1.0
    blob128 = np.ascontiguousarray(np.concatenate([
        pack_bf16(woutT),                                  # 128 cols
        pack_bf16(bias_ones[:, 0:256]),                    # 128 cols
        pack_bf16(np.eye(128, dtype=np.float32)),          # 64 cols
        pack_bf16(bias_ones[:, 256:384]),                  # 64 cols
    ], axis=1, dtype=np.float32))

    in_maps = []
    for core in range(8):
        b, qh = divmod(core, 2)
        xb = x[b].reshape(C, NJ)
        xrot = np.concatenate(
            [xb[:, qh * NI:(qh + 1) * NI], xb[:, (1 - qh) * NI:(2 - qh) * NI]],
            axis=1)
        blob256 = np.concatenate([wqkvT, xrot], axis=1).astype(ml_dtypes.bfloat16)
        in_maps.append({"blob256": np.ascontiguousarray(blob256),
                        "blob128": blob128})
    return in_maps


def run_spmd(x, w_qkv, w_out, b_out, **kw):
    nc = _get_nc()
    in_maps = make_in_maps(x, w_qkv, w_out, b_out)
    return run_bass_kernel_spmd(nc, in_maps, core_ids=list(range(8)), **kw)


def assemble(results):
    out = np.empty((4, C, NJ), np.float32)
    for core in range(8):
        b, qh = divmod(core, 2)
        out[b, :, qh * NI:(qh + 1) * NI] = results[core]["out_t"].T
    return out.reshape(4, C, 64, 64)


def kernel(x, w_qkv, w_out, b_out):
    res = run_spmd(x, w_qkv, w_out, b_out)
    return assemble(res.results)
